# revision 17
# baseline (speedup 1.0000x reference)
"""AdaConv kernel for 8 TRN2 NeuronCores — data-parallel over batch.

Math (verified against the reference, rel err 3e-6):
  The reference's per-sample grouped convs collapse:
    - depthwise conv output is identical across the 8 output channels of each
      group: D[n,g,h,w] = sum_{j,kh,kw} d[n,j,kh,kw] * xpad[n,8g+j,h+kh,w+kw]
    - pointwise 1x1 grouped conv collapses to a per-sample scalar
      S[n] = sum_j (s_d @ pk_w.T + pk_b)[n,j]
    - out = leaky(D[n,c//8]*S[n] + bias[n,c]) * (x - mean)/std  (instance norm)

  On device, per core (2 samples, 8 sample-channel-tiles of 128 channels):
    - stencil: 9 accumulating PE matmuls per pixel window with block-diagonal
      weights W_t[k,m] = d[k%8,t] * (k//8 == m//8); output [128, px] in PSUM is
      already replicated across the 8 channels of each group.
    - ScalarE evicts PSUM with fused leaky(S*D + bias) -> predicted (bf16)
    - instance-norm stats: bf16 full-row accumulates with pad-ring correction
      (all Sqrts batched in one op to avoid ACT table thrashing)
    - GPSIMD does the f32->bf16 cast; DVE does stats accums + final multiply
"""

import numpy as np
import ml_dtypes
from contextlib import ExitStack

import concourse.bass as bass
import concourse.tile as tile
from concourse import bacc, mybir
from concourse.bass_utils import run_bass_kernel_spmd

F32 = mybir.dt.float32
BF16 = mybir.dt.bfloat16
AF = mybir.ActivationFunctionType
ALU = mybir.AluOpType
AX = mybir.AxisListType

N_CORES = 8
NSAMP = 2           # samples per core
CH = 512
H = W = 64
PW = 66             # padded row width/height
PPX = PW * PW       # 4356
PPX_AL = PPX + 4    # padded to 4360 so the last stencil window can over-read
NROWV = 7           # output rows per matmul window (7*66=462 <= 512)
WIN = NROWV * PW    # 462
NWIN_FULL = 9       # 9 windows of 7 rows = rows 0..62; window 9 = row 63
TAP_OFF = [kh * PW + kw for kh in range(3) for kw in range(3)]

LAST_RESULTS = None  # BassKernelResults of the most recent run (for test.py)
_CACHE = {}


def _build():
    nc = bacc.Bacc("TRN2", target_bir_lowering=False, debug=False)

    xpad_d = nc.dram_tensor("xpad", [8, 128, PPX_AL], F32, kind="ExternalInput")
    style_d = nc.dram_tensor("style", [NSAMP, 4, 128, 16], F32, kind="ExternalInput")
    dwT_d = nc.dram_tensor("dwT", [2, 2, 4, 128, 8], F32, kind="ExternalInput")
    dwb_d = nc.dram_tensor("dwb", [8, 1], F32, kind="ExternalInput")
    pbT_d = nc.dram_tensor("pbT", [4, 128, 512], F32, kind="ExternalInput")
    pbb_d = nc.dram_tensor("pbb", [4, 128, 1], F32, kind="ExternalInput")
    pkwT_d = nc.dram_tensor("pkwT", [4, 128, 8], F32, kind="ExternalInput")
    pkb_d = nc.dram_tensor("pkb", [1, 8], F32, kind="ExternalInput")
    mask_d = nc.dram_tensor("mask", [128, 128], BF16, kind="ExternalInput")
    repl8_d = nc.dram_tensor("repl8", [8, 128], F32, kind="ExternalInput")
    out_d = nc.dram_tensor("out", [8, 128, H * W], BF16, kind="ExternalOutput")

    with tile.TileContext(nc) as tc, ExitStack() as ctx:
        const = ctx.enter_context(tc.tile_pool(name="const", bufs=1))
        small = ctx.enter_context(tc.tile_pool(name="small", bufs=1))
        x32p = ctx.enter_context(tc.tile_pool(name="x32", bufs=2))
        x16p = ctx.enter_context(tc.tile_pool(name="x16", bufs=8))
        junkp = ctx.enter_context(tc.tile_pool(name="junk", bufs=1))
        predp = ctx.enter_context(tc.tile_pool(name="pred", bufs=2))
        xnp = ctx.enter_context(tc.tile_pool(name="xn", bufs=2))
        outp = ctx.enter_context(tc.tile_pool(name="outp", bufs=2))
        psum_main = ctx.enter_context(
            tc.tile_pool(name="psum_main", bufs=2, space="PSUM"))
        psum_small = ctx.enter_context(
            tc.tile_pool(name="psum_small", bufs=2, space="PSUM"))

        # ---- load constants / params ----
        mask_sb = const.tile([128, 128], BF16)
        nc.sync.dma_start(mask_sb[:], mask_d[:])
        repl8_sb = const.tile([8, 128], F32)
        nc.sync.dma_start(repl8_sb[:], repl8_d[:])
        dwb_sb = const.tile([8, 1], F32)
        nc.sync.dma_start(dwb_sb[:], dwb_d[:])
        pkb_sb = const.tile([1, 8], F32)
        nc.sync.dma_start(pkb_sb[:], pkb_d[:])
        pbb_sb = const.tile([128, 4], F32)
        for mt in range(4):
            nc.sync.dma_start(pbb_sb[:, mt:mt + 1], pbb_d[mt])
        pbT_sb = const.tile([128, 4, 512], F32)
        for kt in range(4):
            nc.sync.dma_start(pbT_sb[:, kt, :], pbT_d[kt])
        pkwT_sb = const.tile([128, 4, 8], F32)
        for kt in range(4):
            nc.sync.dma_start(pkwT_sb[:, kt, :], pkwT_d[kt])
        style_sb = const.tile([128, NSAMP, 4, 16], F32)
        for s in range(NSAMP):
            for kt in range(4):
                nc.sync.dma_start(style_sb[:, s, kt, :], style_d[s, kt])
        dwT_sb = const.tile([128, 2, 2, 4, 8], F32)
        for ky in range(2):
            for kx in range(2):
                for kt in range(4):
                    nc.sync.dma_start(dwT_sb[:, ky, kx, kt, :], dwT_d[ky, kx, kt])

        # ---- prologue: kernel-predictor math (all tiny, f32) ----
        W_sb = const.tile([128, NSAMP, 9, 128], BF16)     # stencil weights
        bias_sb = const.tile([128, 4, NSAMP], F32)        # per-channel bias [mt, s]
        Sb_sb = const.tile([128, NSAMP], F32)             # S[n] bcast to 128 parts
        d_sb = small.tile([8, NSAMP, 9], F32)
        dcol_sb = small.tile([128, NSAMP, 9], F32)
        ssum_sb = small.tile([128, 4, NSAMP], F32)        # style sums [kt, s]
        pkwsum_sb = small.tile([128, 4], F32)
        pkbsum_sb = small.tile([1, 1], F32)
        S_sb = small.tile([1, NSAMP], F32)

        eps_sb = const.tile([128, 1], F32)
        nc.vector.memset(eps_sb[:], 1e-5)

        nc.vector.tensor_reduce(pkbsum_sb[:], pkb_sb[:], axis=AX.X, op=ALU.add)
        for kt in range(4):
            nc.vector.tensor_reduce(
                pkwsum_sb[:, kt:kt + 1], pkwT_sb[:, kt, :], axis=AX.X, op=ALU.add)

        for s in range(NSAMP):
            # d = leaky(conv2x2(style, dw_w) + dw_b):  16 accumulating matmuls
            ps_d = psum_small.tile([8, 9], F32, tag="ps_sm")
            i = 0
            for ky in range(2):
                for kx in range(2):
                    for kt in range(4):
                        rhs = style_sb[:, s, kt, :].rearrange(
                            "p (y x) -> p y x", x=4)[:, ky:ky + 3, kx:kx + 3]
                        nc.tensor.matmul(
                            ps_d[:], dwT_sb[:, ky, kx, kt, :], rhs,
                            start=(i == 0), stop=(i == 15))
                        i += 1
            nc.scalar.activation(
                d_sb[:, s, :], ps_d[:], AF.Lrelu, bias=dwb_sb[:], alpha=0.01)

            # replicate d over the 128-channel pattern: dcol[c,t] = d[c%8,t]
            ps_dc = psum_small.tile([128, 9], F32, tag="ps_sm")
            nc.tensor.matmul(ps_dc[:], repl8_sb[:], d_sb[:, s, :])
            nc.vector.tensor_copy(dcol_sb[:, s, :], ps_dc[:])

            # stencil weights W_t = mask * dcol[:, t]
            for t in range(9):
                nc.vector.tensor_scalar(
                    W_sb[:, s, t, :], mask_sb[:], dcol_sb[:, s, t:t + 1], None,
                    ALU.mult)

            # style spatial sums (s_d * 16)
            for kt in range(4):
                nc.vector.tensor_reduce(
                    ssum_sb[:, kt, s:s + 1], style_sb[:, s, kt, :],
                    axis=AX.X, op=ALU.add)

        # bias[c] = s_d @ pb_w[c] + pb_b[c]   (both samples batched, N=2)
        for mt in range(4):
            ps_b = psum_small.tile([128, NSAMP], F32, tag="ps_sm")
            for kt in range(4):
                nc.tensor.matmul(
                    ps_b[:], pbT_sb[:, kt, mt * 128:(mt + 1) * 128],
                    ssum_sb[:, kt, :], start=(kt == 0), stop=(kt == 3))
            nc.scalar.activation(
                bias_sb[:, mt, :], ps_b[:], AF.Identity,
                bias=pbb_sb[:, mt:mt + 1], scale=1.0 / 16.0)

        # S = s_d @ pkw_sum + sum(pk_b)   (both samples, N=2)
        ps_S = psum_small.tile([1, NSAMP], F32, tag="ps_sm")
        for kt in range(4):
            nc.tensor.matmul(
                ps_S[:], pkwsum_sb[:, kt:kt + 1], ssum_sb[:, kt, :],
                start=(kt == 0), stop=(kt == 3))
        nc.scalar.activation(
            S_sb[:], ps_S[:], AF.Identity, bias=pkbsum_sb[:], scale=1.0 / 16.0)
        nc.gpsimd.partition_broadcast(Sb_sb[:], S_sb[:])

        # ---- phase A: load + cast + instance-norm statistics ----
        # acc slots per ts: 0 sum(full) 1 sum(rows-ring) 2 sum(cols-ring)
        #                   3 sq(full)  4 sq(rows-ring)  5 sq(cols-ring)
        acc = const.tile([128, 8, 6], F32)
        junkA = junkp.tile([128, PPX], BF16, tag="junkA")
        sqf = junkp.tile([128, PPX], BF16, tag="sqf")
        x16s = []
        for ts in range(8):
            x32 = x32p.tile([128, PPX_AL], F32, tag="x32")
            for c in range(4):  # split the 2.2MB load across DMA queues
                lo, hi = c * 1090, (c + 1) * 1090
                nc.sync.dma_start(x32[:, lo:hi], xpad_d[ts][:, lo:hi])
            x16 = x16p.tile([128, PPX_AL], BF16, tag="x16")
            x16s.append(x16)
            nc.gpsimd.tensor_copy(x16[:], x32[:])

            x16v = x16[:, :PPX].rearrange("p (h w) -> p h w", w=PW)
            rows_ring = x16v[:, 0:PW:PW - 1, :]          # rows 0 and 65
            cols_ring = x16v[:, 1:PW - 1, 0:PW:PW - 1]   # cols 0 and 65 of rows 1..64
            nc.vector.tensor_scalar(
                junkA[:], x16[:, :PPX], 1.0, 0.0, ALU.mult, ALU.add,
                accum_out=acc[:, ts, 0:1])
            nc.vector.tensor_scalar(
                junkA[:, :132].rearrange("p (a b) -> p a b", b=PW), rows_ring,
                1.0, 0.0, ALU.mult, ALU.add, accum_out=acc[:, ts, 1:2])
            nc.vector.tensor_scalar(
                junkA[:, :128].rearrange("p (a b) -> p a b", b=2), cols_ring,
                1.0, 0.0, ALU.mult, ALU.add, accum_out=acc[:, ts, 2:3])

            nc.vector.tensor_tensor(sqf[:], x16[:, :PPX], x16[:, :PPX], ALU.mult)
            sqv = sqf[:].rearrange("p (h w) -> p h w", w=PW)
            nc.vector.tensor_scalar(
                junkA[:], sqf[:], 1.0, 0.0, ALU.mult, ALU.add,
                accum_out=acc[:, ts, 3:4])
            nc.vector.tensor_scalar(
                junkA[:, :132].rearrange("p (a b) -> p a b", b=PW),
                sqv[:, 0:PW:PW - 1, :],
                1.0, 0.0, ALU.mult, ALU.add, accum_out=acc[:, ts, 4:5])
            nc.vector.tensor_scalar(
                junkA[:, :128].rearrange("p (a b) -> p a b", b=2),
                sqv[:, 1:PW - 1, 0:PW:PW - 1],
                1.0, 0.0, ALU.mult, ALU.add, accum_out=acc[:, ts, 5:6])

        # batched stats finalize: all [128, 8], one Sqrt (one ACT table set)
        s_all = small.tile([128, 8], F32)
        nc.vector.tensor_tensor(s_all[:], acc[:, :, 0], acc[:, :, 1], ALU.subtract)
        nc.vector.tensor_tensor(s_all[:], s_all[:], acc[:, :, 2], ALU.subtract)
        q_all = small.tile([128, 8], F32)
        nc.vector.tensor_tensor(q_all[:], acc[:, :, 3], acc[:, :, 4], ALU.subtract)
        nc.vector.tensor_tensor(q_all[:], q_all[:], acc[:, :, 5], ALU.subtract)
        t0_all = small.tile([128, 8], F32)
        nc.vector.tensor_tensor(t0_all[:], s_all[:], s_all[:], ALU.mult)
        u_all = small.tile([128, 8], F32)
        nc.vector.tensor_scalar(
            u_all[:], t0_all[:], -1.0 / 4096.0, None, ALU.mult)
        nc.vector.tensor_tensor(u_all[:], u_all[:], q_all[:], ALU.add)
        stdv_all = small.tile([128, 8], F32)
        nc.scalar.activation(
            stdv_all[:], u_all[:], AF.Sqrt, scale=1.0 / 4095.0, bias=eps_sb[:])
        rstd_all = small.tile([128, 8], F32)
        nc.vector.reciprocal(rstd_all[:], stdv_all[:])
        nmr_all = small.tile([128, 8], F32)
        nc.vector.tensor_tensor(nmr_all[:], rstd_all[:], s_all[:], ALU.mult)
        nc.vector.tensor_scalar(
            nmr_all[:], nmr_all[:], -1.0 / 4096.0, None, ALU.mult)

        # ---- phase B: stencil + predicted + normalize + multiply ----
        for ts in range(8):
            s, q = ts // 4, ts % 4
            x16 = x16s[ts]

            pred = predp.tile([128, H * W], BF16, tag="pred")
            # 10 stencil windows; 3 windows per 3-bank PSUM tile
            for f in range(4):
                wins = range(3 * f, min(3 * f + 3, 10))
                psD = psum_main.tile([128, 1536], F32, tag="psD")
                for slot, w in enumerate(wins):
                    n = WIN if w < NWIN_FULL else PW
                    base = NROWV * PW * w
                    for t in range(9):
                        nc.tensor.matmul(
                            psD[:, 512 * slot: 512 * slot + n],
                            W_sb[:, s, t, :],
                            x16[:, base + TAP_OFF[t]: base + TAP_OFF[t] + n],
                            start=(t == 0), stop=(t == 8))
                # fused leaky(S*D + bias), strided to skip the 2-col seams
                if f < 3:
                    src = psD[:].rearrange("p (k x) -> p k x", x=512)[:, :, :WIN] \
                        .rearrange("p k (r w) -> p k r w", w=PW)[:, :, :, :64]
                    dst = pred[:, 1344 * f: 1344 * (f + 1)]
                else:
                    src = psD[:, :64]
                    dst = pred[:, 4032:4096]
                nc.scalar.activation(
                    dst, src, AF.Lrelu,
                    bias=bias_sb[:, q, s:s + 1], scale=Sb_sb[:, s:s + 1],
                    alpha=0.01)

            # normalized content (bf16): xn = x*rstd + nmr
            xn = xnp.tile([128, H * W], BF16, tag="xn")
            x16c = x16[:, :PPX].rearrange("p (h w) -> p h w", w=PW)[:, 1:65, 1:65]
            nc.scalar.activation(
                xn[:].rearrange("p (h w) -> p h w", w=64), x16c, AF.Identity,
                scale=rstd_all[:, ts:ts + 1], bias=nmr_all[:, ts:ts + 1])

            out_sb = outp.tile([128, H * W], BF16, tag="out")
            nc.vector.tensor_tensor(out_sb[:], pred[:], xn[:], ALU.mult)
            for c in range(2):
                lo, hi = c * 2048, (c + 1) * 2048
                nc.sync.dma_start(out_d[ts][:, lo:hi], out_sb[:, lo:hi])

    nc.compile()
    return nc


def _host_prep(style_encoding, content_in, dw_w, dw_b, pk_w, pk_b, pb_w, pb_b):
    """Shard + lay out inputs for the 8 cores (layout only, no math)."""
    f32 = np.float32
    common = {
        "dwT": np.ascontiguousarray(
            dw_w.transpose(2, 3, 1, 0).reshape(2, 2, 4, 128, 8), f32),
        "dwb": np.ascontiguousarray(dw_b.reshape(8, 1), f32),
        "pbT": np.ascontiguousarray(pb_w.T.reshape(4, 128, 512), f32),
        "pbb": np.ascontiguousarray(pb_b.reshape(4, 128, 1), f32),
        "pkwT": np.ascontiguousarray(pk_w.T.reshape(4, 128, 8), f32),
        "pkb": np.ascontiguousarray(pk_b.reshape(1, 8), f32),
    }
    ii = np.arange(128)
    common["mask"] = ((ii[:, None] // 8) == (ii[None, :] // 8)).astype(
        ml_dtypes.bfloat16)
    common["repl8"] = (np.arange(8)[:, None] == (ii[None, :] % 8)).astype(f32)

    in_maps = []
    for i in range(N_CORES):
        x = content_in[NSAMP * i: NSAMP * (i + 1)]
        xp = np.pad(x, ((0, 0), (0, 0), (1, 1), (1, 1)), mode="reflect")
        xp = xp.reshape(NSAMP, 4, 128, PPX)
        xp = np.concatenate(
            [xp, np.zeros((NSAMP, 4, 128, PPX_AL - PPX), f32)], axis=-1)
        se = style_encoding[NSAMP * i: NSAMP * (i + 1)]
        in_maps.append({
            "xpad": np.ascontiguousarray(xp.reshape(8, 128, PPX_AL), f32),
            "style": np.ascontiguousarray(se.reshape(NSAMP, 4, 128, 16), f32),
            **common,
        })
    return in_maps


def kernel(style_encoding, content_in, dw_w, dw_b, pk_w, pk_b, pb_w, pb_b):
    global LAST_RESULTS
    import os
    if "nc" not in _CACHE:
        _CACHE["nc"] = _build()
    nc = _CACHE["nc"]
    in_maps = _host_prep(style_encoding, content_in, dw_w, dw_b,
                         pk_w, pk_b, pb_w, pb_b)
    res = run_bass_kernel_spmd(
        nc, in_maps, core_ids=list(range(N_CORES)),
        trace=bool(os.environ.get("ADACONV_TRACE")))
    LAST_RESULTS = res
    outs = []
    for i in range(N_CORES):
        o = np.asarray(res.results[i]["out"]).astype(np.float32)
        outs.append(o.reshape(NSAMP, 4, 128, 64, 64).reshape(NSAMP, CH, 64, 64))
    return np.concatenate(outs, axis=0)


# revision 18
# speedup vs baseline: 1.3834x; 1.3834x over previous
"""AdaConv kernel for 8 TRN2 NeuronCores — data-parallel over batch.

Math (verified against the reference, rel err 3e-6):
  The reference's per-sample grouped convs collapse:
    - depthwise conv output is identical across the 8 output channels of each
      group: D[n,g,h,w] = sum_{j,kh,kw} d[n,j,kh,kw] * xpad[n,8g+j,h+kh,w+kw]
    - pointwise 1x1 grouped conv collapses to a per-sample scalar
      S[n] = sum_j (s_d @ pk_w.T + pk_b)[n,j]
    - out = leaky(D[n,c//8]*S[n] + bias[n,c]) * (x - mean)/std  (instance norm)

  On device, per core (2 samples, 8 sample-channel-tiles of 128 channels):
    - stencil: 9 accumulating PE matmuls per pixel window with block-diagonal
      weights W_t[k,m] = d[k%8,t] * (k//8 == m//8); output [128, px] in PSUM is
      already replicated across the 8 channels of each group.
    - ScalarE evicts PSUM with fused leaky(S*D + bias) -> predicted (bf16)
    - instance-norm stats: bf16 full-row accumulates with pad-ring correction
      (all Sqrts batched in one op to avoid ACT table thrashing)
    - GPSIMD does the f32->bf16 cast; DVE does stats accums + final multiply
"""

import numpy as np
import ml_dtypes
from contextlib import ExitStack

import concourse.bass as bass
import concourse.tile as tile
from concourse import bacc, mybir
from concourse.bass_utils import run_bass_kernel_spmd

F32 = mybir.dt.float32
BF16 = mybir.dt.bfloat16
AF = mybir.ActivationFunctionType
ALU = mybir.AluOpType
AX = mybir.AxisListType

N_CORES = 8
NSAMP = 2           # samples per core
CH = 512
H = W = 64
PW = 66             # padded row width/height
PPX = PW * PW       # 4356
PPX_AL = PPX + 4    # padded to 4360 so the last stencil window can over-read
NROWV = 7           # output rows per matmul window (7*66=462 <= 512)
WIN = NROWV * PW    # 462
NWIN_FULL = 9       # 9 windows of 7 rows = rows 0..62; window 9 = row 63
TAP_OFF = [kh * PW + kw for kh in range(3) for kw in range(3)]

LAST_RESULTS = None  # BassKernelResults of the most recent run (for test.py)
_CACHE = {}


def _build():
    nc = bacc.Bacc("TRN2", target_bir_lowering=False, debug=False)

    xpad_d = nc.dram_tensor("xpad", [8, 128, PPX_AL], F32, kind="ExternalInput")
    style_d = nc.dram_tensor("style", [NSAMP, 4, 128, 16], F32, kind="ExternalInput")
    dwT_d = nc.dram_tensor("dwT", [2, 2, 4, 128, 8], F32, kind="ExternalInput")
    dwb_d = nc.dram_tensor("dwb", [8, 1], F32, kind="ExternalInput")
    pbT_d = nc.dram_tensor("pbT", [4, 128, 512], F32, kind="ExternalInput")
    pbb_d = nc.dram_tensor("pbb", [4, 128, 1], F32, kind="ExternalInput")
    pkwT_d = nc.dram_tensor("pkwT", [4, 128, 8], F32, kind="ExternalInput")
    pkb_d = nc.dram_tensor("pkb", [1, 8], F32, kind="ExternalInput")
    mask_d = nc.dram_tensor("mask", [128, 128], BF16, kind="ExternalInput")
    repl8_d = nc.dram_tensor("repl8", [8, 128], F32, kind="ExternalInput")
    out_d = nc.dram_tensor("out", [8, 128, H * W], BF16, kind="ExternalOutput")

    with tile.TileContext(nc) as tc, ExitStack() as ctx:
        const = ctx.enter_context(tc.tile_pool(name="const", bufs=1))
        small = ctx.enter_context(tc.tile_pool(name="small", bufs=1))
        x32p = ctx.enter_context(tc.tile_pool(name="x32", bufs=2))
        x16p = ctx.enter_context(tc.tile_pool(name="x16", bufs=8))
        junkp = ctx.enter_context(tc.tile_pool(name="junk", bufs=1))
        predp = ctx.enter_context(tc.tile_pool(name="pred", bufs=2))
        xnp = ctx.enter_context(tc.tile_pool(name="xn", bufs=2))
        outp = ctx.enter_context(tc.tile_pool(name="outp", bufs=2))
        psum_main = ctx.enter_context(
            tc.tile_pool(name="psum_main", bufs=2, space="PSUM"))
        psum_small = ctx.enter_context(
            tc.tile_pool(name="psum_small", bufs=2, space="PSUM"))

        # ---- load constants / params ----
        mask_sb = const.tile([128, 128], BF16)
        nc.sync.dma_start(mask_sb[:], mask_d[:])
        repl8_sb = const.tile([8, 128], F32)
        nc.sync.dma_start(repl8_sb[:], repl8_d[:])
        dwb_sb = const.tile([8, 1], F32)
        nc.sync.dma_start(dwb_sb[:], dwb_d[:])
        pkb_sb = const.tile([1, 8], F32)
        nc.sync.dma_start(pkb_sb[:], pkb_d[:])
        pbb_sb = const.tile([128, 4], F32)
        for mt in range(4):
            nc.sync.dma_start(pbb_sb[:, mt:mt + 1], pbb_d[mt])
        pbT_sb = const.tile([128, 4, 512], F32)
        for kt in range(4):
            nc.sync.dma_start(pbT_sb[:, kt, :], pbT_d[kt])
        pkwT_sb = const.tile([128, 4, 8], F32)
        for kt in range(4):
            nc.sync.dma_start(pkwT_sb[:, kt, :], pkwT_d[kt])
        style_sb = const.tile([128, NSAMP, 4, 16], F32)
        for s in range(NSAMP):
            for kt in range(4):
                nc.sync.dma_start(style_sb[:, s, kt, :], style_d[s, kt])
        dwT_sb = const.tile([128, 2, 2, 4, 8], F32)
        for ky in range(2):
            for kx in range(2):
                for kt in range(4):
                    nc.sync.dma_start(dwT_sb[:, ky, kx, kt, :], dwT_d[ky, kx, kt])

        # ---- prologue: kernel-predictor math (all tiny, f32) ----
        W_sb = const.tile([128, NSAMP, 9, 128], BF16)     # stencil weights
        bias_sb = const.tile([128, 4, NSAMP], F32)        # per-channel bias [mt, s]
        Sb_sb = const.tile([128, NSAMP], F32)             # S[n] bcast to 128 parts
        d_sb = small.tile([8, NSAMP, 9], F32)
        dcol_sb = small.tile([128, NSAMP, 9], F32)
        ssum_sb = small.tile([128, 4, NSAMP], F32)        # style sums [kt, s]
        pkwsum_sb = small.tile([128, 4], F32)
        pkbsum_sb = small.tile([1, 1], F32)
        S_sb = small.tile([1, NSAMP], F32)

        eps_sb = const.tile([128, 1], F32)
        nc.vector.memset(eps_sb[:], 1e-5)

        nc.vector.tensor_reduce(pkbsum_sb[:], pkb_sb[:], axis=AX.X, op=ALU.add)
        for kt in range(4):
            nc.vector.tensor_reduce(
                pkwsum_sb[:, kt:kt + 1], pkwT_sb[:, kt, :], axis=AX.X, op=ALU.add)

        for s in range(NSAMP):
            # d = leaky(conv2x2(style, dw_w) + dw_b):  16 accumulating matmuls
            ps_d = psum_small.tile([8, 9], F32, tag="ps_sm")
            i = 0
            for ky in range(2):
                for kx in range(2):
                    for kt in range(4):
                        rhs = style_sb[:, s, kt, :].rearrange(
                            "p (y x) -> p y x", x=4)[:, ky:ky + 3, kx:kx + 3]
                        nc.tensor.matmul(
                            ps_d[:], dwT_sb[:, ky, kx, kt, :], rhs,
                            start=(i == 0), stop=(i == 15))
                        i += 1
            nc.scalar.activation(
                d_sb[:, s, :], ps_d[:], AF.Lrelu, bias=dwb_sb[:], alpha=0.01)

            # replicate d over the 128-channel pattern: dcol[c,t] = d[c%8,t]
            ps_dc = psum_small.tile([128, 9], F32, tag="ps_sm")
            nc.tensor.matmul(ps_dc[:], repl8_sb[:], d_sb[:, s, :])
            nc.vector.tensor_copy(dcol_sb[:, s, :], ps_dc[:])

            # stencil weights W_t = mask * dcol[:, t]
            for t in range(9):
                nc.vector.tensor_scalar(
                    W_sb[:, s, t, :], mask_sb[:], dcol_sb[:, s, t:t + 1], None,
                    ALU.mult)

            # style spatial sums (s_d * 16)
            for kt in range(4):
                nc.vector.tensor_reduce(
                    ssum_sb[:, kt, s:s + 1], style_sb[:, s, kt, :],
                    axis=AX.X, op=ALU.add)

        # bias[c] = s_d @ pb_w[c] + pb_b[c]   (both samples batched, N=2)
        for mt in range(4):
            ps_b = psum_small.tile([128, NSAMP], F32, tag="ps_sm")
            for kt in range(4):
                nc.tensor.matmul(
                    ps_b[:], pbT_sb[:, kt, mt * 128:(mt + 1) * 128],
                    ssum_sb[:, kt, :], start=(kt == 0), stop=(kt == 3))
            nc.scalar.activation(
                bias_sb[:, mt, :], ps_b[:], AF.Identity,
                bias=pbb_sb[:, mt:mt + 1], scale=1.0 / 16.0)

        # S = s_d @ pkw_sum + sum(pk_b)   (both samples, N=2)
        ps_S = psum_small.tile([1, NSAMP], F32, tag="ps_sm")
        for kt in range(4):
            nc.tensor.matmul(
                ps_S[:], pkwsum_sb[:, kt:kt + 1], ssum_sb[:, kt, :],
                start=(kt == 0), stop=(kt == 3))
        nc.scalar.activation(
            S_sb[:], ps_S[:], AF.Identity, bias=pkbsum_sb[:], scale=1.0 / 16.0)
        nc.gpsimd.partition_broadcast(Sb_sb[:], S_sb[:])

        # ---- phase A: load + cast + instance-norm statistics ----
        s_all = small.tile([128, 8], F32)     # per-ts sum(x) over center
        q_all = small.tile([128, 8], F32)     # per-ts sum(x^2) over center
        junkA = junkp.tile([128, H * W], BF16, tag="junkA")
        x16s = []
        for ts in range(8):
            x32 = x32p.tile([128, PPX_AL], F32, tag="x32")
            for c in range(4):  # split the 2.2MB load across DMA queues
                lo, hi = c * 1090, (c + 1) * 1090
                nc.sync.dma_start(x32[:, lo:hi], xpad_d[ts][:, lo:hi])
            x16 = x16p.tile([128, PPX_AL], BF16, tag="x16")
            x16s.append(x16)
            nc.vector.tensor_copy(x16[:], x32[:])

            x32c = x32[:, :PPX].rearrange("p (h w) -> p h w", w=PW)[:, 1:65, 1:65]
            nc.vector.tensor_reduce(
                s_all[:, ts:ts + 1], x32c, axis=AX.XY, op=ALU.add)
            nc.scalar.activation(
                junkA[:].rearrange("p (h w) -> p h w", w=64), x32c, AF.Square,
                accum_out=q_all[:, ts:ts + 1])

        # batched stats finalize: all [128, 8], one Sqrt (one ACT table set)
        t0_all = small.tile([128, 8], F32)
        nc.vector.tensor_tensor(t0_all[:], s_all[:], s_all[:], ALU.mult)
        u_all = small.tile([128, 8], F32)
        nc.vector.tensor_scalar(
            u_all[:], t0_all[:], -1.0 / 4096.0, None, ALU.mult)
        nc.vector.tensor_tensor(u_all[:], u_all[:], q_all[:], ALU.add)
        stdv_all = small.tile([128, 8], F32)
        nc.scalar.activation(
            stdv_all[:], u_all[:], AF.Sqrt, scale=1.0 / 4095.0, bias=eps_sb[:])
        rstd_all = small.tile([128, 8], F32)
        nc.vector.reciprocal(rstd_all[:], stdv_all[:])
        nmr_all = small.tile([128, 8], F32)
        nc.vector.tensor_tensor(nmr_all[:], rstd_all[:], s_all[:], ALU.mult)
        nc.vector.tensor_scalar(
            nmr_all[:], nmr_all[:], -1.0 / 4096.0, None, ALU.mult)

        # ---- phase B: stencil + predicted + normalize + multiply ----
        for ts in range(8):
            s, q = ts // 4, ts % 4
            x16 = x16s[ts]

            pred = predp.tile([128, H * W], BF16, tag="pred")
            # 10 stencil windows; 3 windows per 3-bank PSUM tile
            for f in range(4):
                wins = range(3 * f, min(3 * f + 3, 10))
                psD = psum_main.tile([128, 1536], F32, tag="psD")
                for slot, w in enumerate(wins):
                    n = WIN if w < NWIN_FULL else PW
                    base = NROWV * PW * w
                    for t in range(9):
                        nc.tensor.matmul(
                            psD[:, 512 * slot: 512 * slot + n],
                            W_sb[:, s, t, :],
                            x16[:, base + TAP_OFF[t]: base + TAP_OFF[t] + n],
                            start=(t == 0), stop=(t == 8))
                # fused leaky(S*D + bias), strided to skip the 2-col seams
                if f < 3:
                    src = psD[:].rearrange("p (k x) -> p k x", x=512)[:, :, :WIN] \
                        .rearrange("p k (r w) -> p k r w", w=PW)[:, :, :, :64]
                    dst = pred[:, 1344 * f: 1344 * (f + 1)]
                else:
                    src = psD[:, :64]
                    dst = pred[:, 4032:4096]
                nc.scalar.activation(
                    dst, src, AF.Lrelu,
                    bias=bias_sb[:, q, s:s + 1], scale=Sb_sb[:, s:s + 1],
                    alpha=0.01)

            # normalized content (bf16): xn = x*rstd + nmr
            xn = xnp.tile([128, H * W], BF16, tag="xn")
            x16c = x16[:, :PPX].rearrange("p (h w) -> p h w", w=PW)[:, 1:65, 1:65]
            nc.scalar.activation(
                xn[:].rearrange("p (h w) -> p h w", w=64), x16c, AF.Identity,
                scale=rstd_all[:, ts:ts + 1], bias=nmr_all[:, ts:ts + 1])

            out_sb = outp.tile([128, H * W], BF16, tag="out")
            nc.vector.tensor_tensor(out_sb[:], pred[:], xn[:], ALU.mult)
            for c in range(2):
                lo, hi = c * 2048, (c + 1) * 2048
                nc.sync.dma_start(out_d[ts][:, lo:hi], out_sb[:, lo:hi])

    nc.compile()
    return nc


def _host_prep(style_encoding, content_in, dw_w, dw_b, pk_w, pk_b, pb_w, pb_b):
    """Shard + lay out inputs for the 8 cores (layout only, no math)."""
    f32 = np.float32
    common = {
        "dwT": np.ascontiguousarray(
            dw_w.transpose(2, 3, 1, 0).reshape(2, 2, 4, 128, 8), f32),
        "dwb": np.ascontiguousarray(dw_b.reshape(8, 1), f32),
        "pbT": np.ascontiguousarray(pb_w.T.reshape(4, 128, 512), f32),
        "pbb": np.ascontiguousarray(pb_b.reshape(4, 128, 1), f32),
        "pkwT": np.ascontiguousarray(pk_w.T.reshape(4, 128, 8), f32),
        "pkb": np.ascontiguousarray(pk_b.reshape(1, 8), f32),
    }
    ii = np.arange(128)
    common["mask"] = ((ii[:, None] // 8) == (ii[None, :] // 8)).astype(
        ml_dtypes.bfloat16)
    common["repl8"] = (np.arange(8)[:, None] == (ii[None, :] % 8)).astype(f32)

    in_maps = []
    for i in range(N_CORES):
        x = content_in[NSAMP * i: NSAMP * (i + 1)]
        xp = np.pad(x, ((0, 0), (0, 0), (1, 1), (1, 1)), mode="reflect")
        xp = xp.reshape(NSAMP, 4, 128, PPX)
        xp = np.concatenate(
            [xp, np.zeros((NSAMP, 4, 128, PPX_AL - PPX), f32)], axis=-1)
        se = style_encoding[NSAMP * i: NSAMP * (i + 1)]
        in_maps.append({
            "xpad": np.ascontiguousarray(xp.reshape(8, 128, PPX_AL), f32),
            "style": np.ascontiguousarray(se.reshape(NSAMP, 4, 128, 16), f32),
            **common,
        })
    return in_maps


def kernel(style_encoding, content_in, dw_w, dw_b, pk_w, pk_b, pb_w, pb_b):
    global LAST_RESULTS
    import os
    if "nc" not in _CACHE:
        _CACHE["nc"] = _build()
    nc = _CACHE["nc"]
    in_maps = _host_prep(style_encoding, content_in, dw_w, dw_b,
                         pk_w, pk_b, pb_w, pb_b)
    res = run_bass_kernel_spmd(
        nc, in_maps, core_ids=list(range(N_CORES)),
        trace=bool(os.environ.get("ADACONV_TRACE")))
    LAST_RESULTS = res
    outs = []
    for i in range(N_CORES):
        o = np.asarray(res.results[i]["out"]).astype(np.float32)
        outs.append(o.reshape(NSAMP, 4, 128, 64, 64).reshape(NSAMP, CH, 64, 64))
    return np.concatenate(outs, axis=0)


# revision 19
# speedup vs baseline: 1.4635x; 1.0579x over previous
"""AdaConv kernel for 8 TRN2 NeuronCores — data-parallel over batch.

Math (verified against the reference, rel err 3e-6):
  The reference's per-sample grouped convs collapse:
    - depthwise conv output is identical across the 8 output channels of each
      group: D[n,g,h,w] = sum_{j,kh,kw} d[n,j,kh,kw] * xpad[n,8g+j,h+kh,w+kw]
    - pointwise 1x1 grouped conv collapses to a per-sample scalar
      S[n] = sum_j (s_d @ pk_w.T + pk_b)[n,j]
    - out = leaky(D[n,c//8]*S[n] + bias[n,c]) * (x - mean)/std  (instance norm)

  On device, per core (2 samples, 8 sample-channel-tiles of 128 channels):
    - stencil: 9 accumulating PE matmuls per pixel window with block-diagonal
      weights W_t[k,m] = d[k%8,t] * (k//8 == m//8); output [128, px] in PSUM is
      already replicated across the 8 channels of each group.
    - ScalarE evicts PSUM with fused leaky(S*D + bias) -> predicted (bf16)
    - instance-norm stats: bf16 full-row accumulates with pad-ring correction
      (all Sqrts batched in one op to avoid ACT table thrashing)
    - GPSIMD does the f32->bf16 cast; DVE does stats accums + final multiply
"""

import numpy as np
import ml_dtypes
from contextlib import ExitStack

import concourse.bass as bass
import concourse.tile as tile
from concourse import bacc, mybir
from concourse.bass_utils import run_bass_kernel_spmd

F32 = mybir.dt.float32
BF16 = mybir.dt.bfloat16
AF = mybir.ActivationFunctionType
ALU = mybir.AluOpType
AX = mybir.AxisListType

N_CORES = 8
NSAMP = 2           # samples per core
CH = 512
H = W = 64
PW = 66             # padded row width/height
PPX = PW * PW       # 4356
PPX_AL = PPX + 4    # padded to 4360 so the last stencil window can over-read
NROWV = 7           # output rows per matmul window (7*66=462 <= 512)
WIN = NROWV * PW    # 462
NWIN_FULL = 9       # 9 windows of 7 rows = rows 0..62; window 9 = row 63
TAP_OFF = [kh * PW + kw for kh in range(3) for kw in range(3)]

LAST_RESULTS = None  # BassKernelResults of the most recent run (for test.py)
_CACHE = {}


def _build():
    nc = bacc.Bacc("TRN2", target_bir_lowering=False, debug=False)

    xpad_d = nc.dram_tensor("xpad", [8, 128, PPX_AL], F32, kind="ExternalInput")
    style_d = nc.dram_tensor("style", [128, NSAMP, 4, 16], F32, kind="ExternalInput")
    dwT_d = nc.dram_tensor("dwT", [128, 2, 2, 4, 8], F32, kind="ExternalInput")
    dwb_d = nc.dram_tensor("dwb", [8, 1], F32, kind="ExternalInput")
    pbT_d = nc.dram_tensor("pbT", [128, 4, 512], F32, kind="ExternalInput")
    pbb_d = nc.dram_tensor("pbb", [128, 4], F32, kind="ExternalInput")
    pkwT_d = nc.dram_tensor("pkwT", [128, 4, 8], F32, kind="ExternalInput")
    pkb_d = nc.dram_tensor("pkb", [1, 8], F32, kind="ExternalInput")
    mask_d = nc.dram_tensor("mask", [128, 128], BF16, kind="ExternalInput")
    repl8_d = nc.dram_tensor("repl8", [8, 128], F32, kind="ExternalInput")
    out_d = nc.dram_tensor("out", [8, 128, H * W], BF16, kind="ExternalOutput")

    with tile.TileContext(nc) as tc, ExitStack() as ctx:
        const = ctx.enter_context(tc.tile_pool(name="const", bufs=1))
        small = ctx.enter_context(tc.tile_pool(name="small", bufs=1))
        x32p = ctx.enter_context(tc.tile_pool(name="x32", bufs=2))
        x16p = ctx.enter_context(tc.tile_pool(name="x16", bufs=8))
        junkp = ctx.enter_context(tc.tile_pool(name="junk", bufs=1))
        predp = ctx.enter_context(tc.tile_pool(name="pred", bufs=2))
        xnp = ctx.enter_context(tc.tile_pool(name="xn", bufs=2))
        outp = ctx.enter_context(tc.tile_pool(name="outp", bufs=2))
        psum_main = ctx.enter_context(
            tc.tile_pool(name="psum_main", bufs=2, space="PSUM"))
        psum_small = ctx.enter_context(
            tc.tile_pool(name="psum_small", bufs=2, space="PSUM"))

        # ---- first content loads go out before the params ----
        x32s_pre = []
        for ts in range(2):
            x32 = x32p.tile([128, PPX_AL], F32, tag="x32")
            x32s_pre.append(x32)
            for c in range(4):
                lo, hi = c * 1090, (c + 1) * 1090
                nc.sync.dma_start(x32[:, lo:hi], xpad_d[ts][:, lo:hi])

        # ---- constants / params (single coalesced DMA each) ----
        mask_sb = const.tile([128, 128], BF16)
        nc.sync.dma_start(mask_sb[:], mask_d[:])
        repl8_sb = const.tile([8, 128], F32)
        nc.sync.dma_start(repl8_sb[:], repl8_d[:])
        dwb_sb = const.tile([8, 1], F32)
        nc.sync.dma_start(dwb_sb[:], dwb_d[:])
        pkb_sb = const.tile([1, 8], F32)
        nc.sync.dma_start(pkb_sb[:], pkb_d[:])
        pbb_sb = const.tile([128, 4], F32)
        nc.sync.dma_start(pbb_sb[:], pbb_d[:])
        pbT_sb = const.tile([128, 4, 512], F32)
        nc.sync.dma_start(pbT_sb[:], pbT_d[:])
        pkwT_sb = const.tile([128, 4, 8], F32)
        nc.sync.dma_start(pkwT_sb[:], pkwT_d[:])
        style_sb = const.tile([128, NSAMP, 4, 16], F32)
        nc.sync.dma_start(style_sb[:], style_d[:])
        dwT_sb = const.tile([128, 2, 2, 4, 8], F32)
        nc.sync.dma_start(dwT_sb[:], dwT_d[:])

        # ---- prologue: kernel-predictor math (all tiny, f32) ----
        W_sb = const.tile([128, NSAMP, 9, 128], BF16)     # stencil weights
        bias_sb = const.tile([128, 4, NSAMP], F32)        # per-channel bias [mt, s]
        Sb_sb = const.tile([128, NSAMP], F32)             # S[n] bcast to 128 parts
        d_sb = small.tile([8, NSAMP, 9], F32)
        dcol_sb = small.tile([128, NSAMP, 9], F32)
        ssum_sb = small.tile([128, 4, NSAMP], F32)        # style sums [kt, s]
        pkwsum_sb = small.tile([128, 4], F32)
        pkbsum_sb = small.tile([1, 1], F32)
        S_sb = small.tile([1, NSAMP], F32)

        eps_sb = const.tile([128, 1], F32)
        nc.vector.memset(eps_sb[:], 1e-5)

        nc.vector.tensor_reduce(pkbsum_sb[:], pkb_sb[:], axis=AX.X, op=ALU.add)
        for kt in range(4):
            nc.vector.tensor_reduce(
                pkwsum_sb[:, kt:kt + 1], pkwT_sb[:, kt, :], axis=AX.X, op=ALU.add)

        for s in range(NSAMP):
            # d = leaky(conv2x2(style, dw_w) + dw_b):  16 accumulating matmuls
            ps_d = psum_small.tile([8, 9], F32, tag="ps_sm")
            i = 0
            for ky in range(2):
                for kx in range(2):
                    for kt in range(4):
                        rhs = style_sb[:, s, kt, :].rearrange(
                            "p (y x) -> p y x", x=4)[:, ky:ky + 3, kx:kx + 3]
                        nc.tensor.matmul(
                            ps_d[:], dwT_sb[:, ky, kx, kt, :], rhs,
                            start=(i == 0), stop=(i == 15))
                        i += 1
            nc.scalar.activation(
                d_sb[:, s, :], ps_d[:], AF.Lrelu, bias=dwb_sb[:], alpha=0.01)

            # replicate d over the 128-channel pattern: dcol[c,t] = d[c%8,t]
            ps_dc = psum_small.tile([128, 9], F32, tag="ps_sm")
            nc.tensor.matmul(ps_dc[:], repl8_sb[:], d_sb[:, s, :])
            nc.vector.tensor_copy(dcol_sb[:, s, :], ps_dc[:])

            # stencil weights W_t = mask * dcol[:, t]
            for t in range(9):
                nc.vector.tensor_scalar(
                    W_sb[:, s, t, :], mask_sb[:], dcol_sb[:, s, t:t + 1], None,
                    ALU.mult)

            # style spatial sums (s_d * 16)
            for kt in range(4):
                nc.vector.tensor_reduce(
                    ssum_sb[:, kt, s:s + 1], style_sb[:, s, kt, :],
                    axis=AX.X, op=ALU.add)

        # bias[c] = s_d @ pb_w[c] + pb_b[c]   (both samples batched, N=2)
        for mt in range(4):
            ps_b = psum_small.tile([128, NSAMP], F32, tag="ps_sm")
            for kt in range(4):
                nc.tensor.matmul(
                    ps_b[:], pbT_sb[:, kt, mt * 128:(mt + 1) * 128],
                    ssum_sb[:, kt, :], start=(kt == 0), stop=(kt == 3))
            nc.scalar.activation(
                bias_sb[:, mt, :], ps_b[:], AF.Identity,
                bias=pbb_sb[:, mt:mt + 1], scale=1.0 / 16.0)

        # S = s_d @ pkw_sum + sum(pk_b)   (both samples, N=2)
        ps_S = psum_small.tile([1, NSAMP], F32, tag="ps_sm")
        for kt in range(4):
            nc.tensor.matmul(
                ps_S[:], pkwsum_sb[:, kt:kt + 1], ssum_sb[:, kt, :],
                start=(kt == 0), stop=(kt == 3))
        nc.scalar.activation(
            S_sb[:], ps_S[:], AF.Identity, bias=pkbsum_sb[:], scale=1.0 / 16.0)
        nc.gpsimd.partition_broadcast(Sb_sb[:], S_sb[:])

        # ---- phase A: load + cast + instance-norm statistics ----
        s_all = small.tile([128, 8], F32)     # per-ts sum(x) over center
        q_all = small.tile([128, 8], F32)     # per-ts sum(x^2) over center
        junkA = junkp.tile([128, H * W], BF16, tag="junkA")
        x16s = []
        for ts in range(8):
            if ts < 2:
                x32 = x32s_pre[ts]
            else:
                x32 = x32p.tile([128, PPX_AL], F32, tag="x32")
                for c in range(4):  # split the 2.2MB load across DMA queues
                    lo, hi = c * 1090, (c + 1) * 1090
                    nc.sync.dma_start(x32[:, lo:hi], xpad_d[ts][:, lo:hi])
            x16 = x16p.tile([128, PPX_AL], BF16, tag="x16")
            x16s.append(x16)
            nc.vector.tensor_copy(x16[:], x32[:])

            x32c = x32[:, :PPX].rearrange("p (h w) -> p h w", w=PW)[:, 1:65, 1:65]
            nc.vector.tensor_reduce(
                s_all[:, ts:ts + 1], x32c, axis=AX.XY, op=ALU.add)
            nc.scalar.activation(
                junkA[:].rearrange("p (h w) -> p h w", w=64), x32c, AF.Square,
                accum_out=q_all[:, ts:ts + 1])

        # batched stats finalize: all [128, 8], one Sqrt (one ACT table set)
        t0_all = small.tile([128, 8], F32)
        nc.vector.tensor_tensor(t0_all[:], s_all[:], s_all[:], ALU.mult)
        u_all = small.tile([128, 8], F32)
        nc.vector.tensor_scalar(
            u_all[:], t0_all[:], -1.0 / 4096.0, None, ALU.mult)
        nc.vector.tensor_tensor(u_all[:], u_all[:], q_all[:], ALU.add)
        stdv_all = small.tile([128, 8], F32)
        nc.scalar.activation(
            stdv_all[:], u_all[:], AF.Sqrt, scale=1.0 / 4095.0, bias=eps_sb[:])
        rstd_all = small.tile([128, 8], F32)
        nc.vector.reciprocal(rstd_all[:], stdv_all[:])
        nmr_all = small.tile([128, 8], F32)
        nc.vector.tensor_tensor(nmr_all[:], rstd_all[:], s_all[:], ALU.mult)
        nc.vector.tensor_scalar(
            nmr_all[:], nmr_all[:], -1.0 / 4096.0, None, ALU.mult)

        # ---- phase B: stencil + predicted + normalize + multiply ----
        for ts in range(8):
            s, q = ts // 4, ts % 4
            x16 = x16s[ts]

            # normalized content (bf16): xn = x*rstd + nmr
            xn = xnp.tile([128, H * W], BF16, tag="xn")
            x16c = x16[:, :PPX].rearrange("p (h w) -> p h w", w=PW)[:, 1:65, 1:65]
            nc.scalar.activation(
                xn[:].rearrange("p (h w) -> p h w", w=64), x16c, AF.Identity,
                scale=rstd_all[:, ts:ts + 1], bias=nmr_all[:, ts:ts + 1])

            pred = predp.tile([128, H * W], BF16, tag="pred")
            # 10 stencil windows; 3 windows per 3-bank PSUM tile
            for f in range(4):
                wins = range(3 * f, min(3 * f + 3, 10))
                psD = psum_main.tile([128, 1536], F32, tag="psD")
                for slot, w in enumerate(wins):
                    n = WIN if w < NWIN_FULL else PW
                    base = NROWV * PW * w
                    for t in range(9):
                        nc.tensor.matmul(
                            psD[:, 512 * slot: 512 * slot + n],
                            W_sb[:, s, t, :],
                            x16[:, base + TAP_OFF[t]: base + TAP_OFF[t] + n],
                            start=(t == 0), stop=(t == 8))
                # fused leaky(S*D + bias), strided to skip the 2-col seams
                if f < 3:
                    src = psD[:].rearrange("p (k x) -> p k x", x=512)[:, :, :WIN] \
                        .rearrange("p k (r w) -> p k r w", w=PW)[:, :, :, :64]
                    dst = pred[:, 1344 * f: 1344 * (f + 1)]
                else:
                    src = psD[:, :64]
                    dst = pred[:, 4032:4096]
                nc.scalar.activation(
                    dst, src, AF.Lrelu,
                    bias=bias_sb[:, q, s:s + 1], scale=Sb_sb[:, s:s + 1],
                    alpha=0.01)

            out_sb = outp.tile([128, H * W], BF16, tag="out")
            nc.vector.tensor_tensor(out_sb[:], pred[:], xn[:], ALU.mult)
            for c in range(2):
                lo, hi = c * 2048, (c + 1) * 2048
                nc.sync.dma_start(out_d[ts][:, lo:hi], out_sb[:, lo:hi])

    nc.compile()
    return nc


def _host_prep(style_encoding, content_in, dw_w, dw_b, pk_w, pk_b, pb_w, pb_b):
    """Shard + lay out inputs for the 8 cores (layout only, no math)."""
    f32 = np.float32
    common = {
        # dwT[c_lo, ky, kx, kt, p] = dw_w[p, kt*128+c_lo, ky, kx]
        "dwT": np.ascontiguousarray(
            dw_w.reshape(8, 4, 128, 2, 2).transpose(2, 3, 4, 1, 0), f32),
        "dwb": np.ascontiguousarray(dw_b.reshape(8, 1), f32),
        # pbT[c_lo, kt, m] = pb_w[m, kt*128+c_lo]
        "pbT": np.ascontiguousarray(
            pb_w.T.reshape(4, 128, 512).transpose(1, 0, 2), f32),
        "pbb": np.ascontiguousarray(pb_b.reshape(4, 128).T, f32),
        "pkwT": np.ascontiguousarray(
            pk_w.T.reshape(4, 128, 8).transpose(1, 0, 2), f32),
        "pkb": np.ascontiguousarray(pk_b.reshape(1, 8), f32),
    }
    ii = np.arange(128)
    common["mask"] = ((ii[:, None] // 8) == (ii[None, :] // 8)).astype(
        ml_dtypes.bfloat16)
    common["repl8"] = (np.arange(8)[:, None] == (ii[None, :] % 8)).astype(f32)

    in_maps = []
    for i in range(N_CORES):
        x = content_in[NSAMP * i: NSAMP * (i + 1)]
        xp = np.pad(x, ((0, 0), (0, 0), (1, 1), (1, 1)), mode="reflect")
        xp = xp.reshape(NSAMP, 4, 128, PPX)
        xp = np.concatenate(
            [xp, np.zeros((NSAMP, 4, 128, PPX_AL - PPX), f32)], axis=-1)
        se = style_encoding[NSAMP * i: NSAMP * (i + 1)]
        in_maps.append({
            "xpad": np.ascontiguousarray(xp.reshape(8, 128, PPX_AL), f32),
            # style[c_lo, s, kt, yx] = se[s, kt*128+c_lo, yx]
            "style": np.ascontiguousarray(
                se.reshape(NSAMP, 4, 128, 16).transpose(2, 0, 1, 3), f32),
            **common,
        })
    return in_maps


def kernel(style_encoding, content_in, dw_w, dw_b, pk_w, pk_b, pb_w, pb_b):
    global LAST_RESULTS
    import os
    if "nc" not in _CACHE:
        _CACHE["nc"] = _build()
    nc = _CACHE["nc"]
    in_maps = _host_prep(style_encoding, content_in, dw_w, dw_b,
                         pk_w, pk_b, pb_w, pb_b)
    res = run_bass_kernel_spmd(
        nc, in_maps, core_ids=list(range(N_CORES)),
        trace=bool(os.environ.get("ADACONV_TRACE")))
    LAST_RESULTS = res
    outs = []
    for i in range(N_CORES):
        o = np.asarray(res.results[i]["out"]).astype(np.float32)
        outs.append(o.reshape(NSAMP, 4, 128, 64, 64).reshape(NSAMP, CH, 64, 64))
    return np.concatenate(outs, axis=0)
